# revision 22
# baseline (speedup 1.0000x reference)
"""PhiMoE attention (GQA + rotate-half RoPE + sliding-window causal softmax)
TP-sharded across 8 NeuronCores.

Per core c: Q heads 4c..4c+3 (one KV head c), column-sharded o_proj
(output cols c*512:(c+1)*512) fed by an AllGather of per-core attention
outputs. Host concatenates the 8 [2048,512] output shards along axis 1.
"""

import sys

sys.path.insert(0, "/opt/trn_rl_repo")

import numpy as np
from contextlib import ExitStack

B, S, HID = 2, 1024, 4096
NH, NKV, D, G = 32, 8, 128, 4
WINDOW = 512
T = B * S                    # 2048 tokens
SCALE = 1.0 / float(np.sqrt(D))
NCORES = 8
LOC_Q = G * D                # 512 q rows per core
LOC_O = HID // NCORES        # 512 o_proj output cols per core
QKV_LOC = LOC_Q + 2 * D      # 768 fused qkv rows per core

_CACHE = {}


def _build():
    if "nc" in _CACHE:
        return _CACHE["nc"]

    import concourse.bass as bass
    import concourse.bacc as bacc
    import concourse.tile as tile
    from concourse.masks import (
        make_identity,
        make_upper_triangular,
        make_lower_triangular,
    )

    fp32 = bass.mybir.dt.float32
    AF = bass.mybir.ActivationFunctionType

    # Bacc, not Bass: Bacc.finalize() runs the legalization pipeline
    # (generate_event_semaphores etc.) that splits sync waits down to the
    # HW limit of 1 wait per instruction (2 for EventSemaphore).
    nc = bacc.Bacc()
    hT = nc.declare_dram_parameter("hiddenT", [HID, T], fp32, False)
    wqk = nc.declare_dram_parameter("w_qkvT", [HID, QKV_LOC], fp32, False)
    bqk = nc.declare_dram_parameter("b_qkvP", [128, 6], fp32, False)
    cq = nc.declare_dram_parameter("cosq", [D, T], fp32, False)
    sq_ = nc.declare_dram_parameter("sinq", [D, T], fp32, False)
    ck = nc.declare_dram_parameter("cosk", [D, T], fp32, False)
    sk = nc.declare_dram_parameter("sink", [D, T], fp32, False)
    wo = nc.declare_dram_parameter("w_oT", [HID, LOC_O], fp32, False)
    bo = nc.declare_dram_parameter("b_oB", [128, LOC_O], fp32, False)
    out = nc.declare_dram_parameter("out", [T, LOC_O], fp32, True)

    with tile.TileContext(nc) as tc, ExitStack() as stack:
        # ---- persistent constants ----
        misc = stack.enter_context(tc.tile_pool(name="misc", bufs=1))
        identity = misc.tile([128, 128], fp32, name="identity", tag="identity")
        make_identity(nc, identity)
        ones_t = misc.tile([128, 128], fp32, name="ones_t", tag="ones_t")
        nc.vector.memset(ones_t, 1.0)
        cmask = misc.tile([128, 128], fp32, name="cmask", tag="cmask")
        make_upper_triangular(nc, cmask, val=1.0, diag=True)   # 1 where k<=q
        wmask = misc.tile([128, 128], fp32, name="wmask", tag="wmask")
        make_lower_triangular(nc, wmask, val=1.0, diag=False)  # 1 where k>q
        bq_sb = misc.tile([128, 6], fp32, name="bq_sb", tag="bq_sb")
        nc.sync.dma_start(bq_sb, bqk[:, :])
        bo_sb = misc.tile([128, LOC_O], fp32, name="bo_sb", tag="bo_sb")
        nc.sync.dma_start(bo_sb, bo[:, :])
        # half-swap permutation [[0,I64],[I64,0]] (symmetric) for RoPE on PE
        perm = misc.tile([128, 128], fp32, name="perm", tag="perm")
        nc.vector.memset(perm, 0.0)
        nc.sync.dma_start(perm[0:64, 64:128], identity[0:64, 0:64])
        nc.sync.dma_start(perm[64:128, 0:64], identity[0:64, 0:64])

        # ---- persistent qkv tiles: q0..q3, k, v  each [d=128, t=2048] ----
        qkvp = stack.enter_context(tc.tile_pool(name="qkv", bufs=1))
        qkv_sb = [
            qkvp.tile([128, T], fp32, name=f"qkv{i}", tag=f"qkv{i}")
            for i in range(6)
        ]

        # ================= Stage A: fused QKV projection =================
        with ExitStack() as stA:
            wp = stA.enter_context(tc.tile_pool(name="wq", bufs=1))
            w_sb = []
            for h in range(32):
                t = wp.tile([128, QKV_LOC], fp32, name=f"w{h}", tag=f"w{h}")
                nc.sync.dma_start(t, wqk[h * 128:(h + 1) * 128, :])
                w_sb.append(t)
            # bufs=8 so the slot-WAW lands on the same DMA as the 8-back
            # queue-throttle wait (HW DGE DMAs allow at most 2 sync waits).
            hxp = stA.enter_context(tc.tile_pool(name="hx", bufs=8))
            psA = stA.enter_context(tc.tile_pool(name="psA", bufs=1, space="PSUM"))
            for tci in range(4):
                ps = [
                    psA.tile([128, 512], fp32, name=f"psA{o}", tag=f"psA{o}")
                    for o in range(6)
                ]
                for h in range(32):
                    hx = hxp.tile([128, 512], fp32, name="hx", tag="hx")
                    nc.sync.dma_start(
                        hx, hT[h * 128:(h + 1) * 128, tci * 512:(tci + 1) * 512]
                    )
                    for oi in range(6):
                        nc.tensor.matmul(
                            ps[oi],
                            w_sb[h][:, oi * 128:(oi + 1) * 128],
                            hx,
                            start=(h == 0),
                            stop=(h == 31),
                        )
                for oi in range(6):
                    nc.scalar.add(
                        qkv_sb[oi][:, tci * 512:(tci + 1) * 512],
                        ps[oi],
                        bq_sb[:, oi:oi + 1],
                    )

        # ---- persistent mid tiles: v transposed + attention outputs ----
        midp = stack.enter_context(tc.tile_pool(name="mid", bufs=1))
        vp = midp.tile([128, T], fp32, name="vp", tag="vp")
        # one tile so the ag_in store is a single DMA (collective then has
        # only one DMA dependency to wait on)
        attnT3 = midp.tile([128, 4, T], fp32, name="attnT", tag="attnT")
        # virgin-region pool for o_proj input staging: entered here (outer
        # stack) so its SBUF region is never touched by earlier stages and
        # the first agt DMA carries no stale WAW waits.
        agp = stack.enter_context(tc.tile_pool(name="agp", bufs=8))

        # ================= Stage B: RoPE (in place) + V transpose =========
        with ExitStack() as stB:
            csp = stB.enter_context(tc.tile_pool(name="cs", bufs=1))
            cosq_sb = csp.tile([128, T], fp32, name="cosq", tag="cq")
            nc.sync.dma_start(cosq_sb, cq[:, :])
            sinq_sb = csp.tile([128, T], fp32, name="sinq", tag="sq")
            nc.sync.dma_start(sinq_sb, sq_[:, :])
            cosk_sb = csp.tile([128, T], fp32, name="cosk", tag="ck")
            nc.sync.dma_start(cosk_sb, ck[:, :])
            sink_sb = csp.tile([128, T], fp32, name="sink", tag="sk")
            nc.sync.dma_start(sink_sb, sk[:, :])
            tmpp = stB.enter_context(tc.tile_pool(name="tmp", bufs=2))
            ps_r = stB.enter_context(tc.tile_pool(name="ps_r", bufs=2, space="PSUM"))

            # x' = x*cos + half_swap(x)*V, V = [-sin_lo ; sin_hi] (host-built).
            # half_swap = perm @ x on PE (DVE can't mix base partitions and
            # SBUF->SBUF DMAs here exceed the 2-sync-wait DMA limit).
            rope_work = [(qkv_sb[i], cosq_sb, sinq_sb) for i in range(4)]
            rope_work.append((qkv_sb[4], cosk_sb, sink_sb))
            for x, cs, vt in rope_work:
                for c4 in range(4):
                    sl = slice(c4 * 512, (c4 + 1) * 512)
                    psw = ps_r.tile([128, 512], fp32, name="psw", tag="psw")
                    nc.tensor.matmul(psw, perm, x[:, sl], start=True, stop=True)
                    sw = tmpp.tile([128, 512], fp32, name="sw", tag="sw")
                    nc.vector.tensor_mul(sw, psw, vt[:, sl])
                    nc.vector.tensor_mul(x[:, sl], x[:, sl], cs[:, sl])
                    nc.vector.tensor_add(x[:, sl], x[:, sl], sw)

            pst = stB.enter_context(tc.tile_pool(name="pst", bufs=2, space="PSUM"))
            for tt in range(16):
                tp = pst.tile([128, 128], fp32, name="tp", tag="tp")
                nc.tensor.transpose(
                    tp, qkv_sb[5][:, tt * 128:(tt + 1) * 128], identity
                )
                nc.scalar.copy(vp[:, tt * 128:(tt + 1) * 128], tp)

        # ================= Stage C: attention =============================
        with ExitStack() as stC:
            ep = stC.enter_context(tc.tile_pool(name="ep", bufs=16))
            rcp = stC.enter_context(tc.tile_pool(name="rcp", bufs=2))
            ps_s = stC.enter_context(tc.tile_pool(name="ps_s", bufs=2, space="PSUM"))
            ps_pv = stC.enter_context(tc.tile_pool(name="ps_pv", bufs=2, space="PSUM"))
            ps_sm = stC.enter_context(tc.tile_pool(name="ps_sm", bufs=2, space="PSUM"))

            k_t = qkv_sb[4]

            def emit_scores(b, h, qb):
                bcol = b * S
                q_t = qkv_sb[h]
                etiles = {}
                for kt in range(max(0, 4 * qb - 4), 4 * qb + 4):
                    rel = kt - 4 * qb
                    s0 = max(0, rel)
                    s1 = min(3, rel + 4)
                    c0, c1 = 128 * s0, 128 * (s1 + 1)
                    sp = ps_s.tile([128, 512], fp32, name="sp", tag="sp")
                    nc.tensor.matmul(
                        sp[:, c0:c1],
                        k_t[:, bcol + kt * 128: bcol + (kt + 1) * 128],
                        q_t[:, bcol + qb * 512 + c0: bcol + qb * 512 + c1],
                        start=True,
                        stop=True,
                    )
                    e = ep.tile([128, 512], fp32, name="e", tag="e")
                    nc.scalar.activation(e[:, c0:c1], sp[:, c0:c1], AF.Exp)
                    if rel >= 0:  # diagonal sub-tile: causal mask
                        nc.vector.tensor_mul(
                            e[:, rel * 128:(rel + 1) * 128],
                            e[:, rel * 128:(rel + 1) * 128],
                            cmask,
                        )
                    else:  # window-edge sub-tile
                        s = rel + 4
                        nc.vector.tensor_mul(
                            e[:, s * 128:(s + 1) * 128],
                            e[:, s * 128:(s + 1) * 128],
                            wmask,
                        )
                    etiles[kt] = e
                return etiles

            def emit_pv(b, h, qb, etiles):
                bcol = b * S
                for s4 in range(4):
                    m = 4 * qb + s4
                    kts = list(range(max(0, m - 4), m + 1))
                    pv = ps_pv.tile([128, 128], fp32, name="pv", tag="pv")
                    sm = ps_sm.tile([128, 128], fp32, name="sm", tag="sm")
                    for j, kt in enumerate(kts):
                        st, last = (j == 0), (j == len(kts) - 1)
                        ecol = etiles[kt][:, s4 * 128:(s4 + 1) * 128]
                        nc.tensor.matmul(
                            pv,
                            vp[:, (b * 8 + kt) * 128:(b * 8 + kt + 1) * 128],
                            ecol,
                            start=st,
                            stop=last,
                        )
                        nc.tensor.matmul(sm, ones_t, ecol, start=st, stop=last)
                    rc = rcp.tile([128, 128], fp32, name="rc", tag="rc")
                    nc.vector.reciprocal(rc, sm)
                    nc.vector.tensor_mul(
                        attnT3[:, h, bcol + m * 128: bcol + (m + 1) * 128], pv, rc
                    )

            items = [
                (b, h, qb) for b in range(B) for h in range(4) for qb in range(2)
            ]
            prev = None
            for it in items:
                et = emit_scores(*it)
                if prev is not None:
                    emit_pv(*prev[0], prev[1])
                prev = (it, et)
            emit_pv(*prev[0], prev[1])

        # ================= AllGather + Stage D: o_proj ====================
        with ExitStack() as stD:
            dramp = stD.enter_context(tc.tile_pool(name="dramp", bufs=1, space="DRAM"))
            ag_in = dramp.tile([LOC_Q, T], fp32, name="ag_in", tag="agin")
            ag_out = dramp.tile(
                [NCORES * LOC_Q, T], fp32, name="ag_out", tag="agout",
                addr_space="Shared",
            )
            # single DMA: ag_in rows (h*128+p) viewed as (p, h) to match the
            # [128 part, 4, T] attnT tile
            ag_in_v = ag_in[:, :].rearrange("(h p) t -> p h t", h=4)
            nc.sync.dma_start(ag_in_v, attnT3[:, :, :])
            nc.gpsimd.collective_compute(
                "AllGather",
                bass.mybir.AluOpType.bypass,
                replica_groups=[list(range(NCORES))],
                ins=[ag_in.opt()],
                outs=[ag_out.opt()],
            )

            # One big 3D-AP load of all of w_o: single SW-DGE DMA (no throttle
            # chain, off the HW-DGE lane counter), preceded by one DVE memset
            # that absorbs the freed stage-B/C pools' scattered last-touch
            # deps (engine ops tolerate many sync waits; DMAs do not).
            wop = stD.enter_context(tc.tile_pool(name="wop", bufs=1))
            wo_sb3 = wop.tile([128, 32, LOC_O], fp32, name="wo", tag="wo")
            # Pool-engine memset: SW-DGE queue entries execute in Pool program
            # order, so region deps absorbed HERE (engine insts tolerate many
            # waits) are pruned from the following DMA's wait list.
            nc.gpsimd.memset(wo_sb3, 0.0)
            nc.gpsimd.dma_start(wo_sb3, wo.rearrange("(h p) m -> p h m", h=32))
            wo_sb = [wo_sb3[:, hp, :] for hp in range(32)]
            obp = stD.enter_context(tc.tile_pool(name="obp", bufs=2))
            ps_o = stD.enter_context(tc.tile_pool(name="ps_o", bufs=2, space="PSUM"))
            for tg in range(4):
                po = [
                    ps_o.tile([128, LOC_O], fp32, name=f"po{t_}", tag=f"po{t_}")
                    for t_ in range(4)
                ]
                for hp in range(32):
                    agt = agp.tile([128, 512], fp32, name="agt", tag="agt")
                    nc.sync.dma_start(
                        agt,
                        ag_out[hp * 128:(hp + 1) * 128, tg * 512:(tg + 1) * 512],
                    )
                    for tl in range(4):
                        nc.tensor.matmul(
                            po[tl],
                            agt[:, tl * 128:(tl + 1) * 128],
                            wo_sb[hp],
                            start=(hp == 0),
                            stop=(hp == 31),
                        )
                for tl in range(4):
                    ob = obp.tile([128, LOC_O], fp32, name="ob", tag="ob")
                    nc.vector.tensor_add(ob, po[tl], bo_sb)
                    # SW DGE (Pool engine): keeps the HW-DGE lane round-robin a
                    # pure agt stream so slot-WAW and queue-throttle waits
                    # coincide (<=2 sync waits per HW DMA).
                    nc.gpsimd.dma_start(
                        out[(tg * 4 + tl) * 128:(tg * 4 + tl + 1) * 128, :], ob
                    )

    nc.finalize()
    _CACHE["nc"] = nc
    return nc


def _prep(hidden_states, cos, sin, w_qkv, b_qkv, w_o, b_o):
    hs = np.ascontiguousarray(
        hidden_states.reshape(T, HID).T, dtype=np.float32
    )  # [4096, 2048]
    cosT = np.tile(cos.T, (1, B)).astype(np.float32)  # [128, 2048]
    # V = [-sin_lo ; sin_hi]: applied to the half-swapped x on device
    vT = np.tile(
        np.concatenate([-sin[:, :64], sin[:, 64:]], axis=1).T, (1, B)
    ).astype(np.float32)
    cosq = np.ascontiguousarray(cosT * np.float32(SCALE))
    sinq = np.ascontiguousarray(vT * np.float32(SCALE))
    cosk = np.ascontiguousarray(cosT)
    sink = np.ascontiguousarray(vT)
    in_maps = []
    for c in range(NCORES):
        rows = np.r_[
            c * LOC_Q:(c + 1) * LOC_Q,
            NH * D + c * D: NH * D + (c + 1) * D,
            (NH + NKV) * D + c * D: (NH + NKV) * D + (c + 1) * D,
        ]
        wT = np.ascontiguousarray(w_qkv[rows].T, dtype=np.float32)       # [4096,768]
        bP = np.ascontiguousarray(
            b_qkv[rows].reshape(6, 128).T, dtype=np.float32
        )  # [128,6]
        woT = np.ascontiguousarray(
            w_o[c * LOC_O:(c + 1) * LOC_O].T, dtype=np.float32
        )  # [4096,512]
        boB = np.ascontiguousarray(
            np.broadcast_to(b_o[c * LOC_O:(c + 1) * LOC_O], (128, LOC_O)),
            dtype=np.float32,
        )
        in_maps.append(
            dict(
                hiddenT=hs,
                w_qkvT=wT,
                b_qkvP=bP,
                cosq=cosq,
                sinq=sinq,
                cosk=cosk,
                sink=sink,
                w_oT=woT,
                b_oB=boB,
            )
        )
    return in_maps


def run(inputs, trace=False, **kw):
    np_inputs = {k: np.asarray(v, dtype=np.float32) for k, v in inputs.items()}
    nc = _build()
    in_maps = _prep(**np_inputs)
    from concourse.bass_utils import run_bass_kernel_spmd

    res = run_bass_kernel_spmd(nc, in_maps, list(range(NCORES)), trace=trace, **kw)
    out = np.concatenate([res.results[c]["out"] for c in range(NCORES)], axis=1)
    return out.reshape(B, S, HID).astype(np.float32), res


def kernel(**inputs):
    out, _ = run(inputs, trace=False)
    return out


# revision 35
# speedup vs baseline: 2.5690x; 2.5690x over previous
"""PhiMoE attention (GQA + rotate-half RoPE + sliding-window causal softmax)
TP-sharded across 8 NeuronCores.

Per core c: Q heads 4c..4c+3 (one KV head c), column-sharded o_proj
(output cols c*512:(c+1)*512) fed by an AllGather of per-core attention
outputs. Host concatenates the 8 [2048,512] output shards along axis 1.
"""

import sys

sys.path.insert(0, "/opt/trn_rl_repo")

import numpy as np
from contextlib import ExitStack

B, S, HID = 2, 1024, 4096
NH, NKV, D, G = 32, 8, 128, 4
WINDOW = 512
T = B * S                    # 2048 tokens
SCALE = 1.0 / float(np.sqrt(D))
NCORES = 8
LOC_Q = G * D                # 512 q rows per core
LOC_O = HID // NCORES        # 512 o_proj output cols per core
QKV_LOC = LOC_Q + 2 * D      # 768 fused qkv rows per core

_CACHE = {}


def _build():
    if "nc" in _CACHE:
        return _CACHE["nc"]

    import concourse.bass as bass
    import concourse.bacc as bacc
    import concourse.tile as tile
    from concourse.masks import (
        make_identity,
        make_upper_triangular,
        make_lower_triangular,
    )

    fp32 = bass.mybir.dt.float32
    bf16 = bass.mybir.dt.bfloat16
    AF = bass.mybir.ActivationFunctionType

    # Bacc, not Bass: Bacc.finalize() runs the legalization pipeline
    # (generate_event_semaphores etc.) that splits sync waits down to the
    # HW limit of 1 wait per instruction (2 for EventSemaphore).
    nc = bacc.Bacc()
    hT = nc.declare_dram_parameter("hiddenT", [HID, T], bf16, False)
    wqk = nc.declare_dram_parameter("w_qkvT", [HID, QKV_LOC], bf16, False)
    bqk = nc.declare_dram_parameter("b_qkvP", [128, 6], fp32, False)
    cq = nc.declare_dram_parameter("cosq", [D, T], fp32, False)
    sq_ = nc.declare_dram_parameter("sinq", [D, T], fp32, False)
    ck = nc.declare_dram_parameter("cosk", [D, T], fp32, False)
    sk = nc.declare_dram_parameter("sink", [D, T], fp32, False)
    wo = nc.declare_dram_parameter("w_oT", [HID, LOC_O], bf16, False)
    bo = nc.declare_dram_parameter("b_oB", [128, LOC_O], fp32, False)
    out = nc.declare_dram_parameter("out", [T, LOC_O], fp32, True)

    with tile.TileContext(nc) as tc, ExitStack() as stack:
        # ---- persistent constants ----
        misc = stack.enter_context(tc.tile_pool(name="misc", bufs=1))
        identity = misc.tile([128, 128], fp32, name="identity", tag="identity")
        make_identity(nc, identity)
        ones_t = misc.tile([128, 128], fp32, name="ones_t", tag="ones_t")
        nc.vector.memset(ones_t, 1.0)
        cmask = misc.tile([128, 128], fp32, name="cmask", tag="cmask")
        make_upper_triangular(nc, cmask, val=1.0, diag=True)   # 1 where k<=q
        wmask = misc.tile([128, 128], fp32, name="wmask", tag="wmask")
        make_lower_triangular(nc, wmask, val=1.0, diag=False)  # 1 where k>q
        bq_sb = misc.tile([128, 6], fp32, name="bq_sb", tag="bq_sb")
        nc.sync.dma_start(bq_sb, bqk[:, :])
        bo_sb = misc.tile([128, LOC_O], fp32, name="bo_sb", tag="bo_sb")
        nc.sync.dma_start(bo_sb, bo[:, :])
        # half-swap permutation [[0,I64],[I64,0]] (symmetric) for RoPE on PE
        perm = misc.tile([128, 128], fp32, name="perm", tag="perm")
        nc.vector.memset(perm, 0.0)
        nc.sync.dma_start(perm[0:64, 64:128], identity[0:64, 0:64])
        nc.sync.dma_start(perm[64:128, 0:64], identity[0:64, 0:64])

        # ---- persistent qkv tiles: q0..q3, k, v  each [d=128, t=2048] ----
        qkvp = stack.enter_context(tc.tile_pool(name="qkv", bufs=1))
        qkv_sb = [
            qkvp.tile([128, T], fp32, name=f"qkv{i}", tag=f"qkv{i}")
            for i in range(6)
        ]

        # persistent bf16 w_o tile: loaded via SW-DGE right away so the
        # 4MB load fully overlaps stage A (region is virgin, no stale waits)
        wop = stack.enter_context(tc.tile_pool(name="wop", bufs=1))
        wo_sb3 = wop.tile([128, 32, LOC_O], bf16, name="wo", tag="wo")
        nc.gpsimd.dma_start(wo_sb3, wo.rearrange("(h p) m -> p h m", h=32))
        wo_sb = [wo_sb3[:, hp, :] for hp in range(32)]

        # ================= Stage A: fused QKV projection =================
        with ExitStack() as stA:
            wp = stA.enter_context(tc.tile_pool(name="wq", bufs=1))
            w_sb = []
            for h in range(32):
                t = wp.tile([128, QKV_LOC], bf16, name=f"w{h}", tag=f"w{h}")
                nc.sync.dma_start(t, wqk[h * 128:(h + 1) * 128, :])
                w_sb.append(t)
            # bufs=8 so the slot-WAW lands on the same DMA as the 8-back
            # queue-throttle wait (HW DGE DMAs allow at most 2 sync waits).
            hxp = stA.enter_context(tc.tile_pool(name="hx", bufs=8))
            psA = stA.enter_context(tc.tile_pool(name="psA", bufs=1, space="PSUM"))
            for tci in range(4):
                ps = [
                    psA.tile([128, 512], fp32, name=f"psA{o}", tag=f"psA{o}")
                    for o in range(6)
                ]
                for h in range(32):
                    hx = hxp.tile([128, 512], bf16, name="hx", tag="hx")
                    nc.sync.dma_start(
                        hx, hT[h * 128:(h + 1) * 128, tci * 512:(tci + 1) * 512]
                    )
                    for oi in range(6):
                        nc.tensor.matmul(
                            ps[oi],
                            w_sb[h][:, oi * 128:(oi + 1) * 128],
                            hx,
                            start=(h == 0),
                            stop=(h == 31),
                        )
                for oi in range(6):
                    nc.scalar.add(
                        qkv_sb[oi][:, tci * 512:(tci + 1) * 512],
                        ps[oi],
                        bq_sb[:, oi:oi + 1],
                    )

        # ---- persistent mid tiles: v transposed + attention outputs ----
        midp = stack.enter_context(tc.tile_pool(name="mid", bufs=1))
        vp = midp.tile([128, T], fp32, name="vp", tag="vp")
        # one tile so the ag_in store is a single DMA (collective then has
        # only one DMA dependency to wait on)
        attnT3 = midp.tile([128, 4, T], bf16, name="attnT", tag="attnT")
        # virgin-region pool for o_proj input staging: entered here (outer
        # stack) so its SBUF region is never touched by earlier stages and
        # the first agt DMA carries no stale WAW waits.
        agp = stack.enter_context(tc.tile_pool(name="agp", bufs=8))

        # ================= Stage B: RoPE (in place) + V transpose =========
        with ExitStack() as stB:
            csp = stB.enter_context(tc.tile_pool(name="cs", bufs=1))
            cosq_sb = csp.tile([128, T], fp32, name="cosq", tag="cq")
            nc.sync.dma_start(cosq_sb, cq[:, :])
            sinq_sb = csp.tile([128, T], fp32, name="sinq", tag="sq")
            nc.sync.dma_start(sinq_sb, sq_[:, :])
            cosk_sb = csp.tile([128, T], fp32, name="cosk", tag="ck")
            nc.sync.dma_start(cosk_sb, ck[:, :])
            sink_sb = csp.tile([128, T], fp32, name="sink", tag="sk")
            nc.sync.dma_start(sink_sb, sk[:, :])
            tmpp = stB.enter_context(tc.tile_pool(name="tmp", bufs=2))
            ps_r = stB.enter_context(tc.tile_pool(name="ps_r", bufs=2, space="PSUM"))

            # x' = x*cos + half_swap(x)*V, V = [-sin_lo ; sin_hi] (host-built).
            # half_swap = perm @ x on PE (DVE can't mix base partitions and
            # SBUF->SBUF DMAs here exceed the 2-sync-wait DMA limit).
            rope_work = [(qkv_sb[i], cosq_sb, sinq_sb) for i in range(4)]
            rope_work.append((qkv_sb[4], cosk_sb, sink_sb))
            for x, cs, vt in rope_work:
                for c4 in range(4):
                    sl = slice(c4 * 512, (c4 + 1) * 512)
                    psw = ps_r.tile([128, 512], fp32, name="psw", tag="psw")
                    nc.tensor.matmul(psw, perm, x[:, sl], start=True, stop=True)
                    sw = tmpp.tile([128, 512], fp32, name="sw", tag="sw")
                    nc.vector.tensor_mul(sw, psw, vt[:, sl])
                    nc.vector.tensor_mul(x[:, sl], x[:, sl], cs[:, sl])
                    nc.vector.tensor_add(x[:, sl], x[:, sl], sw)

            pst = stB.enter_context(tc.tile_pool(name="pst", bufs=2, space="PSUM"))
            for tt in range(16):
                tp = pst.tile([128, 128], fp32, name="tp", tag="tp")
                nc.tensor.transpose(
                    tp, qkv_sb[5][:, tt * 128:(tt + 1) * 128], identity
                )
                nc.scalar.copy(vp[:, tt * 128:(tt + 1) * 128], tp)

        # ================= Stage C: attention =============================
        with ExitStack() as stC:
            ep = stC.enter_context(tc.tile_pool(name="ep", bufs=16))
            rcp = stC.enter_context(tc.tile_pool(name="rcp", bufs=2))
            ps_s = stC.enter_context(tc.tile_pool(name="ps_s", bufs=2, space="PSUM"))
            ps_pv = stC.enter_context(tc.tile_pool(name="ps_pv", bufs=2, space="PSUM"))
            ps_sm = stC.enter_context(tc.tile_pool(name="ps_sm", bufs=2, space="PSUM"))

            k_t = qkv_sb[4]

            def emit_scores(b, h, qb):
                bcol = b * S
                q_t = qkv_sb[h]
                etiles = {}
                for kt in range(max(0, 4 * qb - 4), 4 * qb + 4):
                    rel = kt - 4 * qb
                    s0 = max(0, rel)
                    s1 = min(3, rel + 4)
                    c0, c1 = 128 * s0, 128 * (s1 + 1)
                    sp = ps_s.tile([128, 512], fp32, name="sp", tag="sp")
                    nc.tensor.matmul(
                        sp[:, c0:c1],
                        k_t[:, bcol + kt * 128: bcol + (kt + 1) * 128],
                        q_t[:, bcol + qb * 512 + c0: bcol + qb * 512 + c1],
                        start=True,
                        stop=True,
                    )
                    e = ep.tile([128, 512], fp32, name="e", tag="e")
                    nc.scalar.activation(e[:, c0:c1], sp[:, c0:c1], AF.Exp)
                    if rel >= 0:  # diagonal sub-tile: causal mask
                        nc.vector.tensor_mul(
                            e[:, rel * 128:(rel + 1) * 128],
                            e[:, rel * 128:(rel + 1) * 128],
                            cmask,
                        )
                    else:  # window-edge sub-tile
                        s = rel + 4
                        nc.vector.tensor_mul(
                            e[:, s * 128:(s + 1) * 128],
                            e[:, s * 128:(s + 1) * 128],
                            wmask,
                        )
                    etiles[kt] = e
                return etiles

            def emit_pv(b, h, qb, etiles):
                bcol = b * S
                for s4 in range(4):
                    m = 4 * qb + s4
                    kts = list(range(max(0, m - 4), m + 1))
                    pv = ps_pv.tile([128, 128], fp32, name="pv", tag="pv")
                    sm = ps_sm.tile([128, 128], fp32, name="sm", tag="sm")
                    for j, kt in enumerate(kts):
                        st, last = (j == 0), (j == len(kts) - 1)
                        ecol = etiles[kt][:, s4 * 128:(s4 + 1) * 128]
                        nc.tensor.matmul(
                            pv,
                            vp[:, (b * 8 + kt) * 128:(b * 8 + kt + 1) * 128],
                            ecol,
                            start=st,
                            stop=last,
                        )
                        nc.tensor.matmul(sm, ones_t, ecol, start=st, stop=last)
                    rc = rcp.tile([128, 128], fp32, name="rc", tag="rc")
                    nc.vector.reciprocal(rc, sm)
                    nc.vector.tensor_mul(
                        attnT3[:, h, bcol + m * 128: bcol + (m + 1) * 128], pv, rc
                    )

            items = [
                (b, h, qb) for b in range(B) for h in range(4) for qb in range(2)
            ]
            prev = None
            for it in items:
                et = emit_scores(*it)
                if prev is not None:
                    emit_pv(*prev[0], prev[1])
                prev = (it, et)
            emit_pv(*prev[0], prev[1])

        # ================= AllGather + Stage D: o_proj ====================
        with ExitStack() as stD:
            dramp = stD.enter_context(tc.tile_pool(name="dramp", bufs=1, space="DRAM"))
            ag_in = dramp.tile([LOC_Q, T], bf16, name="ag_in", tag="agin")
            ag_out = dramp.tile(
                [NCORES * LOC_Q, T], bf16, name="ag_out", tag="agout",
                addr_space="Shared",
            )
            # single DMA: ag_in rows (h*128+p) viewed as (p, h) to match the
            # [128 part, 4, T] attnT tile
            ag_in_v = ag_in[:, :].rearrange("(h p) t -> p h t", h=4)
            nc.sync.dma_start(ag_in_v, attnT3[:, :, :])
            nc.gpsimd.collective_compute(
                "AllGather",
                bass.mybir.AluOpType.bypass,
                replica_groups=[list(range(NCORES))],
                ins=[ag_in.opt()],
                outs=[ag_out.opt()],
            )

            obp = stD.enter_context(tc.tile_pool(name="obp", bufs=2))
            ps_o = stD.enter_context(tc.tile_pool(name="ps_o", bufs=2, space="PSUM"))
            for tg in range(4):
                po = [
                    ps_o.tile([128, LOC_O], fp32, name=f"po{t_}", tag=f"po{t_}")
                    for t_ in range(4)
                ]
                for hp in range(32):
                    agt = agp.tile([128, 512], bf16, name="agt", tag="agt")
                    nc.sync.dma_start(
                        agt,
                        ag_out[hp * 128:(hp + 1) * 128, tg * 512:(tg + 1) * 512],
                    )
                    for tl in range(4):
                        nc.tensor.matmul(
                            po[tl],
                            agt[:, tl * 128:(tl + 1) * 128],
                            wo_sb[hp],
                            start=(hp == 0),
                            stop=(hp == 31),
                        )
                for tl in range(4):
                    ob = obp.tile([128, LOC_O], fp32, name="ob", tag="ob")
                    nc.vector.tensor_add(ob, po[tl], bo_sb)
                    # SW DGE (Pool engine): keeps the HW-DGE lane round-robin a
                    # pure agt stream so slot-WAW and queue-throttle waits
                    # coincide (<=2 sync waits per HW DMA).
                    nc.gpsimd.dma_start(
                        out[(tg * 4 + tl) * 128:(tg * 4 + tl + 1) * 128, :], ob
                    )

    nc.finalize()
    _CACHE["nc"] = nc
    return nc


def _prep(hidden_states, cos, sin, w_qkv, b_qkv, w_o, b_o):
    from ml_dtypes import bfloat16

    hs = np.ascontiguousarray(
        hidden_states.reshape(T, HID).T.astype(bfloat16)
    )  # [4096, 2048] bf16
    cosT = np.tile(cos.T, (1, B)).astype(np.float32)  # [128, 2048]
    # V = [-sin_lo ; sin_hi]: applied to the half-swapped x on device
    vT = np.tile(
        np.concatenate([-sin[:, :64], sin[:, 64:]], axis=1).T, (1, B)
    ).astype(np.float32)
    cosq = np.ascontiguousarray(cosT * np.float32(SCALE))
    sinq = np.ascontiguousarray(vT * np.float32(SCALE))
    cosk = np.ascontiguousarray(cosT)
    sink = np.ascontiguousarray(vT)
    in_maps = []
    for c in range(NCORES):
        rows = np.r_[
            c * LOC_Q:(c + 1) * LOC_Q,
            NH * D + c * D: NH * D + (c + 1) * D,
            (NH + NKV) * D + c * D: (NH + NKV) * D + (c + 1) * D,
        ]
        wT = np.ascontiguousarray(w_qkv[rows].T.astype(bfloat16))       # [4096,768]
        bP = np.ascontiguousarray(
            b_qkv[rows].reshape(6, 128).T, dtype=np.float32
        )  # [128,6]
        woT = np.ascontiguousarray(
            w_o[c * LOC_O:(c + 1) * LOC_O].T.astype(bfloat16)
        )  # [4096,512]
        boB = np.ascontiguousarray(
            np.broadcast_to(b_o[c * LOC_O:(c + 1) * LOC_O], (128, LOC_O)),
            dtype=np.float32,
        )
        in_maps.append(
            dict(
                hiddenT=hs,
                w_qkvT=wT,
                b_qkvP=bP,
                cosq=cosq,
                sinq=sinq,
                cosk=cosk,
                sink=sink,
                w_oT=woT,
                b_oB=boB,
            )
        )
    return in_maps


def run(inputs, trace=False, **kw):
    np_inputs = {k: np.asarray(v, dtype=np.float32) for k, v in inputs.items()}
    nc = _build()
    in_maps = _prep(**np_inputs)
    from concourse.bass_utils import run_bass_kernel_spmd

    res = run_bass_kernel_spmd(nc, in_maps, list(range(NCORES)), trace=trace, **kw)
    out = np.concatenate([res.results[c]["out"] for c in range(NCORES)], axis=1)
    return out.reshape(B, S, HID).astype(np.float32), res


def kernel(**inputs):
    out, _ = run(inputs, trace=False)
    return out
